# revision 12
# baseline (speedup 1.0000x reference)
"""Trainium2 Bass kernel for the CGF tree-GRU problem.

Problem: 3-level complete 8-ary tree GRU (torch GRU cell convention).
  Level 3: 64 nodes x 8 embedded leaf children, h0 = 0
  Level 2:  8 nodes x 8 children (level-3 outputs), h0 = mean of children h
  Level 1:  1 node  x 8 children (level-2 outputs), h0 = mean of children h
  Output: mean over the 8 step outputs of the root GRU. D = 512.

Distribution: one serial chain of 24 GRU steps, each LDW-rate-bound on the
PE; batch/hidden sharding can't pay for per-step collectives.  Replicated
SPMD on 8 cores; core 0's output is returned.

Performance design (v3):
- HAM: the PE runs at 1.2 GHz unless busy ~continuously (then 2.4 GHz), so
  the PE stream is kept dense: per-step PSUM banks accumulate
  bias (identity matmul, start=True) + W_ih@x ("prep", issued during the
  previous step's eltwise tail) + W_hh@h (the recurrent burst).  Sigmoids
  read finished pre-activations straight from PSUM.  This holds for ALL
  levels (the per-step W_ih matmuls for B=8/1 are LDW-bound filler, but
  they keep the PE warm and replace level-transition gi batches).
- The serial per-step chain is the wall:
    gh_r -> sig(r) -> bn=r*gh_n -> ct=bn+gi_n -> tanh -> combine -> h'
  so the combine is shortened algebraically: h' = n*(1-z) + z*h with
  w=1-z and u=z*h computed off-chain right after sig(z); gi_n is drained
  to SBUF bf16 off-chain so ct runs in DVE 2x mode.
- State h is bf16; the output accumulator stays fp32 (GpSimd).
- DMA: 1024-col chunks on the sync+scalar rings (gpsimd's ring trickles),
  ordered by first use so prep(t=0) starts as early as possible.
"""

import numpy as np

import concourse.bacc as bacc
import concourse.mybir as mybir
from concourse.tile import TileContext
from concourse.bass_utils import run_bass_kernel_spmd

AF = mybir.ActivationFunctionType
OP = mybir.AluOpType
FP = mybir.dt.float32
BF = mybir.dt.bfloat16

P = 128          # partitions
D = 512          # hidden size
KT = D // P      # 4 k-tiles (contraction)
G = 3 * D        # 1536 gate dims
MT = G // P      # 12 m-tiles (gate rows)
A = 8            # tree arity == sequence length per level
NB = 64          # level-3 node count
T = 8            # steps per level
N_CORES = 8

# blob16 layout (columns, bf16).  Bias broadcasts are laid out [m(4), b(B)]
# per gate-group and level batch (64/8/1).
_off = 0
def _nxt(n):
    global _off
    o = _off
    _off += n
    return o

O_ID = _nxt(P)                  # identity [P,128]
O_B64 = _nxt(4 * 4 * NB)        # r|z|g|n bias bcast [4][P,4,64]
O_B8 = _nxt(4 * 4 * A)          # r|z|g|n bias bcast [4][P,4,8]
O_B1 = _nxt(4 * 4)              # r|z|g|n bias bcast [4][P,4,1]
O_XT = _nxt(T * KT * NB)        # leaf embeds [P,t,k,node]
O_WIT = _nxt(MT * KT * P)       # W_ih tiles [(m,k) major]
O_WHT = _nxt(MT * KT * P)       # W_hh tiles
B16_COLS = _off

_BUILT = None


def _build_nc():
    nc = bacc.Bacc()

    blob16 = nc.declare_dram_parameter("blob16", [P, B16_COLS], BF, isOutput=False)
    outp = nc.declare_dram_parameter("out", [P, KT], FP, isOutput=True)

    with TileContext(nc) as tc:
        with (
            tc.tile_pool(name="const", bufs=1) as cpool,
            tc.tile_pool(name="state", bufs=1) as spool,
            tc.tile_pool(name="work", bufs=2) as wpool,
            tc.tile_pool(name="pr0", bufs=1, space="PSUM") as pr0,
            tc.tile_pool(name="pr1", bufs=1, space="PSUM") as pr1,
            tc.tile_pool(name="pz0", bufs=1, space="PSUM") as pz0,
            tc.tile_pool(name="pz1", bufs=1, space="PSUM") as pz1,
            tc.tile_pool(name="pn0", bufs=1, space="PSUM") as pn0,
            tc.tile_pool(name="pn1", bufs=1, space="PSUM") as pn1,
            tc.tile_pool(name="pg0", bufs=1, space="PSUM") as pg0,
            tc.tile_pool(name="pg1", bufs=1, space="PSUM") as pg1,
        ):
            # Warm the activation tables first (lazy ACT_TABLE_LOADs
            # otherwise land mid-kernel and stall sigmoids by >1us).
            warm = cpool.tile([P, 8], FP)
            nc.vector.memset(warm[:, :], 0.0)
            for fn in (AF.Identity, AF.Sigmoid, AF.Tanh):
                nc.scalar.activation(warm[:, :], warm[:, :], fn)
            ones = cpool.tile([P, 4 * NB], BF)
            nc.vector.memset(ones[:, :], 1.0)

            # Chunked input DMA on the two proven HWDGE rings, ordered by
            # first use; consumers gate on the chunks covering their slices.
            b16_sb = cpool.tile([P, B16_COLS], BF)
            ranges = []
            CH = 1024
            for c in range(0, O_XT, CH):                     # misc + biases
                ranges.append((c, min(c + CH, O_XT)))
            ranges.append((O_XT, O_XT + 512))                # xt t0-t1
            for c in range(O_WIT, O_WHT, CH):                # wit
                ranges.append((c, min(c + CH, O_WHT)))
            for c in range(O_WHT, B16_COLS, CH):             # wht
                ranges.append((c, min(c + CH, B16_COLS)))
            for c in range(O_XT + 512, O_WIT, CH):           # xt rest
                ranges.append((c, min(c + CH, O_WIT)))
            for i, (c0, c1) in enumerate(ranges):
                eng = nc.sync if i % 2 == 0 else nc.scalar
                eng.dma_start(out=b16_sb[:, c0:c1], in_=blob16[:, c0:c1])

            ident = b16_sb[:, O_ID:O_ID + P]

            def bias_ap(lvl, gate):  # gate 0=r 1=z 2=g(b_in) 3=n(b_hn)
                base, b = {3: (O_B64, NB), 2: (O_B8, A), 1: (O_B1, 1)}[lvl]
                c = base + gate * 4 * b
                return b16_sb[:, c:c + 4 * b]

            xt = b16_sb[:, O_XT:O_XT + T * KT * NB]
            xtv = xt.rearrange("p (t k b) -> p t k b", t=T, k=KT)

            def wit(m, k):
                c = O_WIT + (m * KT + k) * P
                return b16_sb[:, c:c + P]

            def wht(m, k):
                c = O_WHT + (m * KT + k) * P
                return b16_sb[:, c:c + P]

            def mm(dst, lhsT, rhs, start, stop):
                nc.tensor.matmul(dst, lhsT=lhsT, rhs=rhs, start=start,
                                 stop=stop, skip_group_check=True)

            # ---------------- generic level runner ----------------
            def run_level(lvl, B, h0_tile, x_rhs_of_tk):
                """8 GRU steps.  h0_tile: [P, KT*B] bf16 initial state (None
                for level 3 => zeros and no t0 recurrent burst).
                x_rhs_of_tk(t, k) -> [P, B] bf16 rhs for the prep matmuls.
                Returns (h_final, acc)."""
                rp = (pr0, pr1)
                zp = (pz0, pz1)
                np_ = (pn0, pn1)
                gp = (pg0, pg1)
                sfx = f"L{lvl}"
                W = 4 * B
                hA = spool.tile([P, KT * B], BF, tag=f"hA{sfx}")
                hB = spool.tile([P, KT * B], BF, tag=f"hB{sfx}")
                acc = spool.tile([P, KT * B], FP, tag=f"acc{sfx}")
                if lvl == 3:
                    hz = spool.tile([P, KT * B], BF, tag=f"hz{sfx}")
                    nc.vector.memset(hz[:, :], 0.0)

                banks = {}

                def get_banks(t):
                    par = t & 1
                    if par not in banks or banks[par][0] != t:
                        pR = rp[par].tile([P, 512], FP, tag=f"r{par}",
                                          name=f"pR{par}{sfx}")
                        pZ = zp[par].tile([P, 512], FP, tag=f"z{par}",
                                          name=f"pZ{par}{sfx}")
                        pN = np_[par].tile([P, 512], FP, tag=f"n{par}",
                                           name=f"pN{par}{sfx}")
                        pG = gp[par].tile([P, 512], FP, tag=f"g{par}",
                                          name=f"pG{par}{sfx}")
                        gin = wpool.tile([P, W], BF, tag=f"gin{par}{sfx}",
                                         name=f"gin{par}{sfx}")
                        banks[par] = (t, pR, pZ, pN, pG, gin)
                    return banks[par][1:]

                def emit_prep(t, final):
                    """bias id-preloads + W_ih@x(t) into the parity banks,
                    then drain gi_n+b_in to SBUF (off the critical chain)."""
                    pR, pZ, pN, pG, gin = get_banks(t)
                    for dst, gate, mlo in ((pR, 0, 0), (pZ, 1, 4), (pG, 2, 8)):
                        mm(dst[:, :W], ident, bias_ap(lvl, gate),
                           start=True, stop=False)
                        for mi in range(4):
                            m = mlo + mi
                            for k in range(KT):
                                mm(dst[:, mi * B:(mi + 1) * B], wit(m, k),
                                   x_rhs_of_tk(t, k), start=False,
                                   stop=(mi == 3 and k == KT - 1
                                         and (final or dst is pG)))
                    mm(pN[:, :W], ident, bias_ap(lvl, 3), start=True,
                       stop=final)

                def emit_drain(t):
                    # gi_n + b_in: PSUM -> SBUF bf16, off the critical chain
                    # (must be emitted AFTER the current step's ACT ops --
                    # the ACT queue is strict FIFO).
                    _, _, _, pG, gin = get_banks(t)
                    nc.scalar.activation(gin[:, :], pG[:, :W], AF.Identity)

                def emit_gh(t, h_prev):
                    pR, pZ, pN, pG, gin = get_banks(t)
                    for dst, mlo in ((pR, 0), (pN, 8), (pZ, 4)):
                        for mi in range(4):
                            m = mlo + mi
                            for k in range(KT):
                                mm(dst[:, mi * B:(mi + 1) * B], wht(m, k),
                                   h_prev[:, k * B:(k + 1) * B], start=False,
                                   stop=mi == 3 and k == KT - 1)

                h_prev = h0_tile if lvl != 3 else hz
                emit_prep(0, final=lvl == 3)
                for t in range(T):
                    pR, pZ, pN, pG, gin = get_banks(t)
                    if lvl != 3 or t > 0:
                        emit_gh(t, h_prev)
                    if t + 1 < T:
                        emit_prep(t + 1, final=False)
                    # ---- eltwise tail ----
                    # chain: sig(r) -> bn -> ct -> tanh -> v -> h16
                    # off-chain: sig(z) -> w = 1-z, u = z*h_prev
                    rt = wpool.tile([P, W], BF, tag=f"rt{sfx}")
                    nc.scalar.activation(rt[:, :], pR[:, :W], AF.Sigmoid)
                    if t == 0:
                        emit_drain(0)
                    bnw = wpool.tile([P, W], BF, tag=f"bn{sfx}")
                    nc.vector.tensor_mul(bnw[:, :], rt[:, :], pN[:, :W])
                    ct = wpool.tile([P, W], BF, tag=f"ct{sfx}")
                    nc.vector.tensor_add(ct[:, :], bnw[:, :], gin[:, :])
                    zt = wpool.tile([P, W], BF, tag=f"zt{sfx}")
                    nc.scalar.activation(zt[:, :], pZ[:, :W], AF.Sigmoid)
                    nt = wpool.tile([P, W], BF, tag=f"nt{sfx}")
                    nc.scalar.activation(nt[:, :], ct[:, :], AF.Tanh)
                    wt = wpool.tile([P, W], BF, tag=f"wt{sfx}")
                    nc.vector.tensor_sub(wt[:, :], ones[:, :W], zt[:, :])
                    ut = wpool.tile([P, W], BF, tag=f"ut{sfx}")
                    nc.vector.tensor_mul(ut[:, :], zt[:, :], h_prev[:, :])
                    vt = wpool.tile([P, W], BF, tag=f"vt{sfx}")
                    nc.vector.tensor_mul(vt[:, :], nt[:, :], wt[:, :])
                    h_new = hA if t & 1 == 0 else hB
                    nc.vector.tensor_add(h_new[:, :], vt[:, :], ut[:, :])
                    if t == 0:
                        nc.gpsimd.tensor_copy(acc[:, :], h_new[:, :])
                    else:
                        nc.gpsimd.tensor_add(acc[:, :], acc[:, :], h_new[:, :])
                    if t + 1 < T:
                        emit_drain(t + 1)
                    h_prev = h_new

                return h_prev, acc

            # ================= LEVEL 3 =================
            h3, acc3 = run_level(3, NB, None, lambda t, k: xtv[:, t, k])

            # ---------------- Level 3 -> 2 transition ----------------
            # x2[p, k, t, j] = acc3[p, k, j, t] / 8   (j = parent)
            x2 = spool.tile([P, KT * NB], BF, tag="x2")
            x2v = x2[:].rearrange("p (k t j) -> p k t j", k=KT, t=A)
            acc3p = acc3[:].rearrange("p (k j t) -> p k t j", k=KT, j=A)
            nc.scalar.mul(x2v, acc3p, 1.0 / A)
            hr2 = spool.tile([P, KT * A], FP, tag="hr2")
            nc.vector.tensor_reduce(
                hr2[:].rearrange("p (k j) -> p k j", k=KT),
                h3[:].rearrange("p (k j t) -> p k j t", k=KT, j=A),
                axis=mybir.AxisListType.X, op=OP.add)
            h02 = spool.tile([P, KT * A], BF, tag="h02")
            nc.scalar.mul(h02[:, :], hr2[:, :], 1.0 / A)

            x2r = x2[:].rearrange("p (k c) -> p k c", k=KT)
            h2, acc2 = run_level(
                2, A, h02, lambda t, k: x2r[:, k, t * A:(t + 1) * A])

            # ---------------- Level 2 -> 1 transition ----------------
            x1 = spool.tile([P, KT * A], BF, tag="x1")
            nc.scalar.mul(x1[:, :], acc2[:, :], 1.0 / A)
            hr1 = spool.tile([P, KT], FP, tag="hr1")
            nc.vector.tensor_reduce(
                hr1[:].rearrange("p (k j) -> p k j", k=KT),
                h2[:].rearrange("p (k j t) -> p k j t", k=KT, j=1),
                axis=mybir.AxisListType.X, op=OP.add)
            h01 = spool.tile([P, KT], BF, tag="h01")
            nc.scalar.mul(h01[:, :], hr1[:, :], 1.0 / A)

            x1r = x1[:].rearrange("p (k c) -> p k c", k=KT)
            h1, acc1 = run_level(
                1, 1, h01, lambda t, k: x1r[:, k, t:t + 1])

            out_sb = spool.tile([P, KT], FP, tag="outsb")
            nc.scalar.mul(out_sb[:, :], acc1[:, :], 1.0 / A)
            nc.sync.dma_start(out=outp[:, :], in_=out_sb[:, :])

    nc.finalize()
    return nc


def _get_nc():
    global _BUILT
    if _BUILT is None:
        _BUILT = _build_nc()
    return _BUILT


def make_inputs(leaf_ids, embed_table, W_ih, W_hh, b_ih, b_hh):
    """Host-side layout prep: gather looked-up embedding rows and lay all
    tensors out in the on-chip transposed format (one bf16 blob)."""
    import ml_dtypes

    leaf_ids = np.asarray(leaf_ids).astype(np.int64)
    emb = np.asarray(embed_table, dtype=np.float32)
    W_ih = np.asarray(W_ih, dtype=np.float32)
    W_hh = np.asarray(W_hh, dtype=np.float32)
    b_ih = np.asarray(b_ih, dtype=np.float32)
    b_hh = np.asarray(b_hh, dtype=np.float32)

    x = emb[leaf_ids]                              # [node(64), t(8), 512]
    xt = np.ascontiguousarray(
        x.reshape(NB, T, KT, P).transpose(3, 1, 2, 0)).reshape(P, -1)

    def pack_w(W):  # [1536, 512] -> [P, (m,k) tiles]
        WT = np.ascontiguousarray(W.T)             # [512, 1536]
        return np.ascontiguousarray(
            WT.reshape(KT, P, MT, P).transpose(1, 2, 0, 3)).reshape(P, -1)

    ident = np.eye(P, dtype=np.float32)

    gb12 = np.concatenate([(b_ih + b_hh)[:2 * D], b_ih[2 * D:]]).reshape(MT, P)
    bhn4 = b_hh[2 * D:].reshape(KT, P)

    def bcast(rows, b):                            # rows [4, P] -> [P, 4*b]
        return np.repeat(rows.T[:, :, None], b, axis=2).reshape(P, -1)

    def bias_group(b):
        return np.concatenate(
            [bcast(gb12[0:4], b), bcast(gb12[4:8], b),
             bcast(gb12[8:12], b), bcast(bhn4, b)], axis=1)

    blob16 = np.concatenate(
        [ident, bias_group(NB), bias_group(A), bias_group(1),
         xt, pack_w(W_ih), pack_w(W_hh)], axis=1).astype(ml_dtypes.bfloat16)
    assert blob16.shape == (P, B16_COLS), blob16.shape
    return {"blob16": np.ascontiguousarray(blob16)}


def unpack_output(out_np):
    # out [P, KT]: element (p, k) = root dim k*128+p
    return np.ascontiguousarray(out_np.T).reshape(1, 1, D).astype(np.float32)


def kernel(leaf_ids=None, layer=None, embed_table=None, W_ih=None, W_hh=None,
           b_ih=None, b_hh=None, **_unused):
    in_map = make_inputs(leaf_ids, embed_table, W_ih, W_hh, b_ih, b_hh)
    nc = _get_nc()
    res = run_bass_kernel_spmd(nc, [in_map] * N_CORES, list(range(N_CORES)))
    return unpack_output(res.results[0]["out"])
